# revision 3
# baseline (speedup 1.0000x reference)
"""Trainium2 Bass kernel for nn_AudioVisualModel audio-visual contrastive loss.

Strategy (8 NeuronCores, SPMD), V3 — two-compute-engine pipeline + DMA offload:
  - Shard the visual batch axis: core m owns y in {2m, 2m+1}. Every core gets
    the full (normalized, transposed) audio features plus its own visual
    shard (~6 MB/core input HBM traffic).
  - Host: L2-normalize both inputs (fp32), lay audio out as AT[k][128d,
    2048tok] and visual as VT[k][128d, 3920] where each 490-column bank chunk
    is t-major: col-in-bank = t*49 + j with v = b*49 + j, so every SBUF
    access downstream is innermost-packed (16B cacheline friendly).
  - Device, per (x, yl) slab (32 slabs):
      PE  : 8 fp32r matmuls (K=2x128, N=490) into a 4-bank PSUM slab
      ACT : one Copy pass PSUM fp32 -> SBUF bf16 (xb) -- the only PSUM read
      DVE : custom dual-stream op AV_MINSQ2 (min(x,0)*x + min(y,0)*y,
            accum=add) over the xb halves -> per-slab nonneg partial in one
            980-cycle pass
      DVE : tensor_tensor(max) xb halves (bf16 packed, 2x mode) -> y1
            [128, 980] = max over bank pairs
      DMA : y1 -> DRAM (2 chunks/slab, spread over the 16 DMA engines,
            hidden under compute). The remaining max over (2 x 49) elements
            per (token, t) runs on the host -- that offload is what lets the
            DVE keep pace with ACT.
  - Host: bf16 y1 -> max over (pair, j) -> per-(slab, t) clip partials;
    partition sums, 16x16 InfoNCE softmax, temperature scaling (max/mean/
    min-square commute with the positive temperature divide).
"""
import sys

sys.path.insert(0, "/opt/trn_rl_repo")

import numpy as np

B, NA, T, NV, D = 16, 128, 10, 196, 256
N_CORES = 8
Y_PER_CORE = B // N_CORES          # 2
COLS_PER_Y = T * NV                # 1960
N_SLABS = B * Y_PER_CORE           # 32 per core
BANKW = 512                        # fp32 psum bank width
CHUNK = 490                        # 10 t * 49 j per bank
JB = 49                            # v-groups per bank (v = b*49 + j)
NBANK = 4                          # banks per slab (4*490 = 1960)
HCH = 2 * CHUNK                    # 980: half-slab (bank pair) width
Y1_COLS = N_SLABS * HCH            # 31360 bf16 cols in the y1 output

_PROG_CACHE = {}
_MINSQ2 = None


def _get_minsq2():
    """Register (once per process) a custom dual-stream DVE op:
        out[k]    = min(in0[k],0)*in0[k] + min(in1[k],0)*in1[k]
        accum_out = sum_k out[k]
    Reads two packed SBUF streams per cycle -> the whole per-slab nonneg
    reduction in one 980-cycle instruction."""
    global _MINSQ2
    if _MINSQ2 is not None:
        return _MINSQ2
    from operator import add as _add

    from concourse import dve_ops
    from concourse.dve_spec import Spec, Src0, Src1, Zero, lower, minn
    from concourse.dve_uop import DveOpSpec

    name = "AV_MINSQ2"
    for op in dve_ops.OPS:
        if op.name == name:
            _MINSQ2 = op
            return op

    def _ref(in0, in1, c0, c1, c2):
        x0 = np.asarray(in0, np.float32)
        x1 = np.asarray(in1, np.float32)
        body = (np.minimum(x0, 0.0) * x0 + np.minimum(x1, 0.0) * x1).astype(
            np.float32
        )
        return body, body.reshape(body.shape[0], -1).sum(axis=-1, keepdims=True)

    spec = Spec(
        body=minn(Src0, Zero) * Src0 + minn(Src1, Zero) * Src1,
        accum=_add,
        reference=_ref,
    )
    row = max(dve_ops._SUB_OPCODE_FOR_NAME.values()) + 1
    assert row < 0x20
    shas = {}
    for ver in ("v3", "v4"):
        uops = lower(spec, ver=ver)
        shas[ver] = DveOpSpec(name=name, opcode=row, uops=uops, rd1_en=True).sha(
            ver
        )
    op = dve_ops.DveOp(name, spec, subdim=False, uops_sha=shas)
    dve_ops.OPS.append(op)
    dve_ops.CUSTOM_DVE_SPECS[name] = spec
    dve_ops._SUB_OPCODE_FOR_NAME[name] = row
    _MINSQ2 = op
    return op


def _build_program(mm_dtype_name="float32r", loop_reps=1):
    """loop_reps > 1 wraps the compute pass in a hardware loop (used only by
    the timing harness to measure per-iteration HW time differentially)."""
    import contextlib

    import concourse.tile as tile
    from concourse import bacc, mybir

    minsq2 = _get_minsq2()
    mm_dt = getattr(mybir.dt, mm_dtype_name)
    f32 = mybir.dt.float32
    bf16 = mybir.dt.bfloat16

    nc = bacc.Bacc("TRN2", target_bir_lowering=False, debug=False,
                   num_devices=N_CORES)
    at_d = nc.declare_dram_parameter("at", [2, 128, 2048], mm_dt, isOutput=False)
    vt_d = nc.declare_dram_parameter("vt", [2, 128, 2 * COLS_PER_Y], mm_dt,
                                     isOutput=False)
    y1_d = nc.declare_dram_parameter("y1o", [128, Y1_COLS], bf16, isOutput=True)
    nn_d = nc.declare_dram_parameter("nno", [128, N_SLABS], f32, isOutput=True)

    with tile.TileContext(nc) as tc:
        with (
            tc.tile_pool(name="persist", bufs=1) as pp,
            tc.tile_pool(name="scratch", bufs=2) as zp,
            tc.tile_pool(name="y1pool", bufs=4) as yp,
            tc.tile_pool(name="psum", bufs=2, space="PSUM") as ps,
        ):
            # Persistent input tiles, chunked so DMA deps stay fine-grained.
            at_t = [[pp.tile([128, 512], mm_dt, name=f"at{k}_{g}",
                             tag=f"at{k}_{g}") for g in range(4)]
                    for k in range(2)]
            vt_t = [[[pp.tile([128, CHUNK], mm_dt, name=f"vt{k}_{yl}_{b}",
                              tag=f"vt{k}_{yl}_{b}") for b in range(NBANK)]
                     for yl in range(2)]
                    for k in range(2)]
            nn = pp.tile([128, N_SLABS], f32, name="nn", tag="nn")
            dummy = pp.tile([128, 1], f32, name="dummy", tag="dummy")

            # Tiny activation issued first so the ACT function-table load
            # happens during the DMA lead-in, off the critical path.
            nc.vector.memset(dummy[:], 0.0)
            nc.scalar.activation(out=dummy[:], in_=dummy[:],
                                 func=mybir.ActivationFunctionType.Copy)

            # DMA issue order tracks first use.
            nc.sync.dma_start(at_t[0][0][:], at_d[0, :, 0:512])
            nc.sync.dma_start(at_t[1][0][:], at_d[1, :, 0:512])
            for b in range(NBANK):
                for k in range(2):
                    nc.sync.dma_start(
                        vt_t[k][0][b][:],
                        vt_d[k, :, b * CHUNK:(b + 1) * CHUNK])
            for g in range(1, 4):
                for k in range(2):
                    nc.sync.dma_start(
                        at_t[k][g][:], at_d[k, :, g * 512:(g + 1) * 512])
            for b in range(NBANK):
                for k in range(2):
                    nc.sync.dma_start(
                        vt_t[k][1][b][:],
                        vt_d[k, :, COLS_PER_Y + b * CHUNK:
                             COLS_PER_Y + (b + 1) * CHUNK])

            if loop_reps > 1:
                loop_cm = tc.For_i(0, loop_reps, 1,
                                   hint_engines=(mybir.EngineType.PE,))
            else:
                loop_cm = contextlib.nullcontext()
            loop_stack = contextlib.ExitStack()
            loop_stack.enter_context(loop_cm)

            for i in range(N_SLABS):
                yl, x = divmod(i, B)
                slab = ps.tile([128, NBANK * BANKW], f32, name=f"slab{i}",
                               tag="slab")
                for k in range(2):
                    lhsT = at_t[k][x // 4][:, (x % 4) * 128:(x % 4 + 1) * 128]
                    for b in range(NBANK):
                        nc.tensor.matmul(
                            slab[:, b * BANKW:b * BANKW + CHUNK],
                            lhsT=lhsT,
                            rhs=vt_t[k][yl][b][:, 0:CHUNK],
                            start=(k == 0), stop=(k == 1))

                # ACT: the one PSUM read -- fp32 -> bf16 copy into SBUF.
                xb = zp.tile([128, NBANK * CHUNK], bf16, name=f"xb_{i}",
                             tag="xb")
                banks = slab[:].rearrange("p (b w) -> p b w", b=NBANK)[:, :, 0:CHUNK]
                nc.scalar.activation(
                    out=xb[:].rearrange("p (b c) -> p b c", b=NBANK),
                    in_=banks,
                    func=mybir.ActivationFunctionType.Copy)

                h0 = xb[:, 0:HCH]
                h1 = xb[:, HCH:2 * HCH]

                # DVE: whole nonneg reduction in one dual-stream pass.
                ms = zp.tile([128, HCH], bf16, name=f"ms_{i}", tag="ms")
                nc.vector._custom_dve(minsq2, out=ms[:], in0=h0, in1=h1,
                                      accum_out=nn[:, i:i + 1])

                # DVE: max over bank pairs (bf16 packed -> 2x mode).
                y1 = yp.tile([128, HCH], bf16, name=f"y1_{i}", tag="y1")
                nc.vector.tensor_tensor(out=y1[:], in0=h0, in1=h1,
                                        op=mybir.AluOpType.max)

                # Ship y1 to DRAM in two chunks (separate DMA queues).
                for c in range(2):
                    nc.sync.dma_start(
                        y1_d[:, i * HCH + c * CHUNK:i * HCH + (c + 1) * CHUNK],
                        y1[:, c * CHUNK:(c + 1) * CHUNK])

            loop_stack.close()
            nc.sync.dma_start(nn_d[:], nn[:])

    nc.compile()
    return nc


def _get_program(mm_dtype_name="float32r", loop_reps=1):
    key = (mm_dtype_name, loop_reps)
    if key not in _PROG_CACHE:
        _PROG_CACHE[key] = _build_program(mm_dtype_name, loop_reps)
    return _PROG_CACHE[key]


def _prep_inputs(audio_feats, visual_feats):
    a = np.ascontiguousarray(np.asarray(audio_feats, dtype=np.float32))
    v = np.ascontiguousarray(np.asarray(visual_feats, dtype=np.float32))
    an = a / np.maximum(
        np.sqrt((a * a).sum(-1, keepdims=True, dtype=np.float32)), 1e-12)
    vn = v / np.maximum(
        np.sqrt((v * v).sum(-1, keepdims=True, dtype=np.float32)), 1e-12)

    # AT[k, d, tok]; tok = x*128 + a_tok, d split as k*128 + dd (d-major)
    at = np.ascontiguousarray(
        an.reshape(B * NA, 2, 128).transpose(1, 2, 0))
    in_maps = []
    for m in range(N_CORES):
        vloc = vn[2 * m:2 * m + 2]                      # (2, T, NV, D)
        # col = yl*1960 + b*490 + t*49 + j  with v = b*49 + j
        vt = vloc.reshape(2, T, NBANK, JB, 2, 128)       # (yl,t,b,j,k,dd)
        vt = vt.transpose(4, 5, 0, 2, 1, 3)              # (k,dd,yl,b,t,j)
        vt = np.ascontiguousarray(vt).reshape(2, 128, 2 * COLS_PER_Y)
        in_maps.append({"at": at, "vt": vt})
    return in_maps


def _bf16_to_f32(arr):
    a = np.asarray(arr)
    if a.dtype == np.float32:
        return a
    if a.dtype.itemsize == 2 and a.dtype.kind in "uiV":
        return (a.view(np.uint16).astype(np.uint32) << 16).view(np.float32)
    return a.astype(np.float32)


def _finalize(core_outs, temperature):
    """core_outs: list of 8 dicts with y1o [128, 31360] bf16 and
    nno [128, 32] fp32. Host-side final max + gather."""
    Tf = float(temperature)
    clip = np.zeros((B, B), dtype=np.float64)
    nonneg_sum = 0.0
    for m, out in enumerate(core_outs):
        y1 = _bf16_to_f32(out["y1o"]).reshape(128, N_SLABS, 2, T, JB)
        tmax = y1.max(axis=(2, 4))                       # [128, 32, T]
        tmsum = tmax.sum(axis=0, dtype=np.float64)       # [32, T]
        tmsum = tmsum.reshape(2, B, T)                   # [yl, x, t]
        clip[:, 2 * m] = tmsum[0].sum(axis=1)
        clip[:, 2 * m + 1] = tmsum[1].sum(axis=1)
        nonneg_sum += np.asarray(out["nno"], np.float64).sum()

    clip /= (NA * T)            # mean over audio tokens and time
    clip /= Tf                  # temperature (commutes with max/mean)

    # InfoNCE on the diagonal
    def log_softmax_diag(mat):
        mx = mat.max(axis=1, keepdims=True)
        lse = np.log(np.exp(mat - mx).sum(axis=1)) + mx[:, 0]
        return np.diag(mat) - lse

    losses = -(log_softmax_diag(clip) + log_softmax_diag(clip.T))
    contrastive = 0.5 * losses.mean()

    l_nonneg = nonneg_sum / (B * B * NA * T * NV) / (Tf * Tf)
    log_t = np.log(Tf)
    temp_low = max(-log_t, 0.0) ** 4
    temp_high = max(log_t - np.log(3.0), 0.0) ** 4
    reg = l_nonneg + temp_low + temp_high
    total = contrastive + 0.3 * reg
    return (np.float32(total), np.float32(contrastive), np.float32(reg))


def kernel(audio_feats, visual_feats, temperature):
    from concourse.bass_utils import run_bass_kernel_spmd

    nc = _get_program()
    in_maps = _prep_inputs(audio_feats, visual_feats)
    res = run_bass_kernel_spmd(nc, in_maps, list(range(N_CORES)))
    core_outs = [res.results[m] for m in range(N_CORES)]
    return _finalize(core_outs, temperature)


# revision 18
# speedup vs baseline: 1.0220x; 1.0220x over previous
"""Trainium2 Bass kernel for nn_AudioVisualModel audio-visual contrastive loss.

Strategy (8 NeuronCores, SPMD), V3 — two-compute-engine pipeline + DMA offload:
  - Shard the visual batch axis: core m owns y in {2m, 2m+1}. Every core gets
    the full (normalized, transposed) audio features plus its own visual
    shard (~6 MB/core input HBM traffic).
  - Host: L2-normalize both inputs (fp32), lay audio out as AT[k][128d,
    2048tok] and visual as VT[k][128d, 3920] where each 490-column bank chunk
    is t-major: col-in-bank = t*49 + j with v = b*49 + j, so every SBUF
    access downstream is innermost-packed (16B cacheline friendly).
  - Device, per (x, yl) slab (32 slabs):
      PE  : 8 fp32r matmuls (K=2x128, N=490) into a 4-bank PSUM slab
      ACT : one Copy pass PSUM fp32 -> SBUF bf16 (xb) -- the only PSUM read
      DVE : custom dual-stream op AV_MINSQ2 (min(x,0)*x + min(y,0)*y,
            accum=add) over the xb halves -> per-slab nonneg partial in one
            980-cycle pass
      DVE : tensor_tensor(max) xb halves (bf16 packed, 2x mode) -> y1
            [128, 980] = max over bank pairs
      DMA : y1 -> DRAM (2 chunks/slab, spread over the 16 DMA engines,
            hidden under compute). The remaining max over (2 x 49) elements
            per (token, t) runs on the host -- that offload is what lets the
            DVE keep pace with ACT.
  - Host: bf16 y1 -> max over (pair, j) -> per-(slab, t) clip partials;
    partition sums, 16x16 InfoNCE softmax, temperature scaling (max/mean/
    min-square commute with the positive temperature divide).
"""
import sys

sys.path.insert(0, "/opt/trn_rl_repo")

import numpy as np

B, NA, T, NV, D = 16, 128, 10, 196, 256
N_CORES = 8
Y_PER_CORE = B // N_CORES          # 2
COLS_PER_Y = T * NV                # 1960
N_SLABS = B * Y_PER_CORE           # 32 per core
BANKW = 512                        # fp32 psum bank width
CHUNK = 490                        # 10 t * 49 j per bank
JB = 49                            # v-groups per bank (v = b*49 + j)
NBANK = 4                          # banks per slab (4*490 = 1960)
HCH = 2 * CHUNK                    # 980: half-slab (bank pair) width
Y1_COLS = N_SLABS * HCH            # 31360 bf16 cols in the y1 output

_PROG_CACHE = {}
_MINSQ2 = None

# PE filler (HW-swept 2026-08-08: pure overhead, the PE p-state ramp is NOT
# the bottleneck mechanism here -- keep 0).
FILL_COLS = 0

# fp8 quantization scale for the normalized features: lifts the bulk of the
# distribution (sigma ~= 1/16) out of e4m3's subnormal range. sims come back
# scaled by FP8_SCALE**2; the host divides it out.
FP8_SCALE = 16.0


def _get_minsq2():
    """Register (once per process) a custom dual-stream DVE op:
        out[k]    = min(in0[k],0)*in0[k] + min(in1[k],0)*in1[k]
        accum_out = sum_k out[k]
    Reads two packed SBUF streams per cycle -> the whole per-slab nonneg
    reduction in one 980-cycle instruction."""
    global _MINSQ2
    if _MINSQ2 is not None:
        return _MINSQ2
    from operator import add as _add

    from concourse import dve_ops
    from concourse.dve_spec import Spec, Src0, Src1, Zero, lower, minn
    from concourse.dve_uop import DveOpSpec

    name = "AV_MINSQ2"
    for op in dve_ops.OPS:
        if op.name == name:
            _MINSQ2 = op
            return op

    def _ref(in0, in1, c0, c1, c2):
        x0 = np.asarray(in0, np.float32)
        x1 = np.asarray(in1, np.float32)
        body = (np.minimum(x0, 0.0) * x0 + np.minimum(x1, 0.0) * x1).astype(
            np.float32
        )
        return body, body.reshape(body.shape[0], -1).sum(axis=-1, keepdims=True)

    spec = Spec(
        body=minn(Src0, Zero) * Src0 + minn(Src1, Zero) * Src1,
        accum=_add,
        reference=_ref,
    )
    row = max(dve_ops._SUB_OPCODE_FOR_NAME.values()) + 1
    assert row < 0x20
    shas = {}
    for ver in ("v3", "v4"):
        uops = lower(spec, ver=ver)
        shas[ver] = DveOpSpec(name=name, opcode=row, uops=uops, rd1_en=True).sha(
            ver
        )
    op = dve_ops.DveOp(name, spec, subdim=False, uops_sha=shas)
    dve_ops.OPS.append(op)
    dve_ops.CUSTOM_DVE_SPECS[name] = spec
    dve_ops._SUB_OPCODE_FOR_NAME[name] = row
    _MINSQ2 = op
    return op


def _build_program(mm_dtype_name="float32r", loop_reps=1, fill_cols=None):
    """loop_reps > 1 wraps the compute pass in a hardware loop (used only by
    the timing harness to measure per-iteration HW time differentially)."""
    import contextlib

    import concourse.tile as tile
    from concourse import bacc, mybir

    if fill_cols is None:
        fill_cols = FILL_COLS

    minsq2 = _get_minsq2()
    mm_dt = getattr(mybir.dt, mm_dtype_name)
    f32 = mybir.dt.float32
    bf16 = mybir.dt.bfloat16

    is_fp8 = mm_dtype_name.startswith("float8")
    nc = bacc.Bacc("TRN2", target_bir_lowering=False, debug=False,
                   num_devices=N_CORES)
    if is_fp8:
        # (kt, tok)-major audio / (yl, b, kt, col)-major visual, fp8.
        at_d = nc.declare_dram_parameter("at", [128, 2 * 2048], mm_dt,
                                         isOutput=False)
        vt_d = nc.declare_dram_parameter("vt", [128, 2 * NBANK * 2 * CHUNK],
                                         mm_dt, isOutput=False)
    else:
        at_d = nc.declare_dram_parameter("at", [2, 128, 2048], mm_dt,
                                         isOutput=False)
        vt_d = nc.declare_dram_parameter("vt", [2, 128, 2 * COLS_PER_Y], mm_dt,
                                         isOutput=False)
    y1_d = nc.declare_dram_parameter("y1o", [128, Y1_COLS], bf16, isOutput=True)
    nn_d = nc.declare_dram_parameter("nno", [128, N_SLABS], f32, isOutput=True)

    with tile.TileContext(nc) as tc:
        with (
            tc.tile_pool(name="persist", bufs=1) as pp,
            tc.tile_pool(name="scratch", bufs=2) as zp,
            tc.tile_pool(name="y1pool", bufs=4) as yp,
            tc.tile_pool(name="psum", bufs=2, space="PSUM") as ps,
        ):
            # Persistent input tiles, chunked so DMA deps stay fine-grained.
            if is_fp8:
                at8 = [pp.tile([128, 1024], mm_dt, name=f"at8_{g}",
                               tag=f"at8_{g}") for g in range(4)]
                vt8 = [[pp.tile([128, 2 * CHUNK], mm_dt, name=f"vt8_{yl}_{b}",
                                tag=f"vt8_{yl}_{b}") for b in range(NBANK)]
                       for yl in range(2)]
            else:
                at_t = [[pp.tile([128, 512], mm_dt, name=f"at{k}_{g}",
                                 tag=f"at{k}_{g}") for g in range(4)]
                        for k in range(2)]
                vt_t = [[[pp.tile([128, CHUNK], mm_dt, name=f"vt{k}_{yl}_{b}",
                                  tag=f"vt{k}_{yl}_{b}") for b in range(NBANK)]
                         for yl in range(2)]
                        for k in range(2)]
            nn = pp.tile([128, N_SLABS], f32, name="nn", tag="nn")
            dummy = pp.tile([128, 1], f32, name="dummy", tag="dummy")
            zt = pp.tile([128, 128], mm_dt, name="zt", tag="zt")

            # Tiny activation issued first so the ACT function-table load
            # happens during the DMA lead-in, off the critical path.
            nc.vector.memset(dummy[:], 0.0)
            nc.scalar.activation(out=dummy[:], in_=dummy[:],
                                 func=mybir.ActivationFunctionType.Copy)
            nc.vector.memset(zt[:].bitcast(mybir.dt.uint32), 0)

            # DMA issue order tracks first use.
            if is_fp8:
                nc.sync.dma_start(at8[0][:], at_d[:, 0:1024])
                for b in range(NBANK):
                    nc.sync.dma_start(
                        vt8[0][b][:],
                        vt_d[:, b * 2 * CHUNK:(b + 1) * 2 * CHUNK])
                for g in range(1, 4):
                    nc.sync.dma_start(at8[g][:],
                                      at_d[:, g * 1024:(g + 1) * 1024])
                for b in range(NBANK):
                    nc.sync.dma_start(
                        vt8[1][b][:],
                        vt_d[:, NBANK * 2 * CHUNK + b * 2 * CHUNK:
                             NBANK * 2 * CHUNK + (b + 1) * 2 * CHUNK])
            else:
                nc.sync.dma_start(at_t[0][0][:], at_d[0, :, 0:512])
                nc.sync.dma_start(at_t[1][0][:], at_d[1, :, 0:512])
                for b in range(NBANK):
                    for k in range(2):
                        nc.sync.dma_start(
                            vt_t[k][0][b][:],
                            vt_d[k, :, b * CHUNK:(b + 1) * CHUNK])
                for g in range(1, 4):
                    for k in range(2):
                        nc.sync.dma_start(
                            at_t[k][g][:], at_d[k, :, g * 512:(g + 1) * 512])
                for b in range(NBANK):
                    for k in range(2):
                        nc.sync.dma_start(
                            vt_t[k][1][b][:],
                            vt_d[k, :, COLS_PER_Y + b * CHUNK:
                                 COLS_PER_Y + (b + 1) * CHUNK])

            if loop_reps > 1:
                loop_cm = tc.For_i(0, loop_reps, 1,
                                   hint_engines=(mybir.EngineType.PE,))
            else:
                loop_cm = contextlib.nullcontext()
            loop_stack = contextlib.ExitStack()
            loop_stack.enter_context(loop_cm)

            for i in range(N_SLABS):
                yl, x = divmod(i, B)
                slab = ps.tile([128, NBANK * BANKW], f32, name=f"slab{i}",
                               tag="slab")
                if is_fp8:
                    # DoubleRow: both 128-deep k-tiles in one pass, 0.5
                    # cycles/row.
                    lhsT = at8[x // 4][:, (x % 4) * 256:(x % 4 + 1) * 256]
                    lhsT = lhsT.rearrange("p (kt m) -> p kt m", kt=2)
                    for b in range(NBANK):
                        rhs = vt8[yl][b][:].rearrange(
                            "p (kt n) -> p kt n", kt=2)
                        nc.tensor.matmul(
                            slab[:, b * BANKW:b * BANKW + CHUNK],
                            lhsT=lhsT, rhs=rhs, start=True, stop=True,
                            perf_mode=mybir.MatmulPerfMode.DoubleRow)
                else:
                    for k in range(2):
                        lhsT = at_t[k][x // 4][:,
                                               (x % 4) * 128:(x % 4 + 1) * 128]
                        for b in range(NBANK):
                            nc.tensor.matmul(
                                slab[:, b * BANKW:b * BANKW + CHUNK],
                                lhsT=lhsT,
                                rhs=vt_t[k][yl][b][:, 0:CHUNK],
                                start=(k == 0), stop=(k == 1))
                # zero-accumulate fillers: keep the PE stream busy (HW-swept:
                # not helpful on this part, fill_cols=0).
                nf = fill_cols
                b = 0
                while nf > 0:
                    w = max(min(nf, CHUNK), 256)
                    nc.tensor.matmul(
                        slab[:, b * BANKW:b * BANKW + w],
                        lhsT=zt[:],
                        rhs=(vt8[yl][b] if is_fp8 else vt_t[0][yl][b])[:, 0:w],
                        start=False, stop=(nf - w <= 0),
                        skip_group_check=True)
                    nf -= w
                    b = (b + 1) % NBANK

                # ACT: the one PSUM read -- fp32 -> bf16 copy into SBUF.
                xb = zp.tile([128, NBANK * CHUNK], bf16, name=f"xb_{i}",
                             tag="xb")
                banks = slab[:].rearrange("p (b w) -> p b w", b=NBANK)[:, :, 0:CHUNK]
                nc.scalar.activation(
                    out=xb[:].rearrange("p (b c) -> p b c", b=NBANK),
                    in_=banks,
                    func=mybir.ActivationFunctionType.Copy)

                h0 = xb[:, 0:HCH]
                h1 = xb[:, HCH:2 * HCH]

                # DVE: whole nonneg reduction in one dual-stream pass.
                ms = zp.tile([128, HCH], bf16, name=f"ms_{i}", tag="ms")
                nc.vector._custom_dve(minsq2, out=ms[:], in0=h0, in1=h1,
                                      accum_out=nn[:, i:i + 1])

                # DVE: max over bank pairs (bf16 packed -> 2x mode).
                y1 = yp.tile([128, HCH], bf16, name=f"y1_{i}", tag="y1")
                nc.vector.tensor_tensor(out=y1[:], in0=h0, in1=h1,
                                        op=mybir.AluOpType.max)

                # Ship y1 to DRAM in two chunks (separate DMA queues).
                for c in range(2):
                    nc.sync.dma_start(
                        y1_d[:, i * HCH + c * CHUNK:i * HCH + (c + 1) * CHUNK],
                        y1[:, c * CHUNK:(c + 1) * CHUNK])

            loop_stack.close()
            nc.sync.dma_start(nn_d[:], nn[:])

    nc.compile()
    return nc


def _get_program(mm_dtype_name="float32r", loop_reps=1):
    key = (mm_dtype_name, loop_reps)
    if key not in _PROG_CACHE:
        _PROG_CACHE[key] = _build_program(mm_dtype_name, loop_reps)
    return _PROG_CACHE[key]


def _prep_inputs(audio_feats, visual_feats, mm_dtype_name="float32r"):
    a = np.ascontiguousarray(np.asarray(audio_feats, dtype=np.float32))
    v = np.ascontiguousarray(np.asarray(visual_feats, dtype=np.float32))
    an = a / np.maximum(
        np.sqrt((a * a).sum(-1, keepdims=True, dtype=np.float32)), 1e-12)
    vn = v / np.maximum(
        np.sqrt((v * v).sum(-1, keepdims=True, dtype=np.float32)), 1e-12)

    if mm_dtype_name.startswith("float8"):
        import ml_dtypes

        f8 = (ml_dtypes.float8_e4m3fn if mm_dtype_name == "float8e4"
              else ml_dtypes.float8_e5m2)
        an8 = (an * FP8_SCALE).astype(f8)
        vn8 = (vn * FP8_SCALE).astype(f8)
        # at[p, x, kt, m] = an[x, m, kt*128+p];  col = x*256 + kt*128 + m
        at = an8.reshape(B, NA, 2, 128).transpose(3, 0, 2, 1)
        at = np.ascontiguousarray(at).reshape(128, 2 * 2048)
        in_maps = []
        for m in range(N_CORES):
            vloc = vn8[2 * m:2 * m + 2]                  # (2, T, NV, D)
            # col = yl*3920 + b*980 + kt*490 + t*49 + j
            vt = vloc.reshape(2, T, NBANK, JB, 2, 128)   # (yl,t,b,j,kt,p)
            vt = vt.transpose(5, 0, 2, 4, 1, 3)          # (p,yl,b,kt,t,j)
            vt = np.ascontiguousarray(vt).reshape(128, 2 * NBANK * 2 * CHUNK)
            in_maps.append({"at": at, "vt": vt})
        return in_maps

    if mm_dtype_name == "bfloat16":
        import ml_dtypes

        an = an.astype(ml_dtypes.bfloat16)
        vn = vn.astype(ml_dtypes.bfloat16)

    # AT[k, d, tok]; tok = x*128 + a_tok, d split as k*128 + dd (d-major)
    at = np.ascontiguousarray(
        an.reshape(B * NA, 2, 128).transpose(1, 2, 0))
    in_maps = []
    for m in range(N_CORES):
        vloc = vn[2 * m:2 * m + 2]                      # (2, T, NV, D)
        # col = yl*1960 + b*490 + t*49 + j  with v = b*49 + j
        vt = vloc.reshape(2, T, NBANK, JB, 2, 128)       # (yl,t,b,j,k,dd)
        vt = vt.transpose(4, 5, 0, 2, 1, 3)              # (k,dd,yl,b,t,j)
        vt = np.ascontiguousarray(vt).reshape(2, 128, 2 * COLS_PER_Y)
        in_maps.append({"at": at, "vt": vt})
    return in_maps


def _bf16_to_f32(arr):
    a = np.asarray(arr)
    if a.dtype == np.float32:
        return a
    if a.dtype.itemsize == 2 and a.dtype.kind in "uiV":
        return (a.view(np.uint16).astype(np.uint32) << 16).view(np.float32)
    return a.astype(np.float32)


def _finalize(core_outs, temperature, sim_scale=1.0):
    """core_outs: list of 8 dicts with y1o [128, 31360] bf16 and
    nno [128, 32] fp32. Host-side final max + gather. sim_scale is the
    factor by which device sims are scaled up (fp8 quantization scale^2)."""
    Tf = float(temperature)
    clip = np.zeros((B, B), dtype=np.float64)
    nonneg_sum = 0.0
    for m, out in enumerate(core_outs):
        y1 = _bf16_to_f32(out["y1o"]).reshape(128, N_SLABS, 2, T, JB)
        tmax = y1.max(axis=(2, 4))                       # [128, 32, T]
        tmsum = tmax.sum(axis=0, dtype=np.float64)       # [32, T]
        tmsum = tmsum.reshape(2, B, T)                   # [yl, x, t]
        clip[:, 2 * m] = tmsum[0].sum(axis=1)
        clip[:, 2 * m + 1] = tmsum[1].sum(axis=1)
        nonneg_sum += np.asarray(out["nno"], np.float64).sum()

    clip /= (NA * T)            # mean over audio tokens and time
    clip /= sim_scale
    nonneg_sum /= sim_scale * sim_scale
    clip /= Tf                  # temperature (commutes with max/mean)

    # InfoNCE on the diagonal
    def log_softmax_diag(mat):
        mx = mat.max(axis=1, keepdims=True)
        lse = np.log(np.exp(mat - mx).sum(axis=1)) + mx[:, 0]
        return np.diag(mat) - lse

    losses = -(log_softmax_diag(clip) + log_softmax_diag(clip.T))
    contrastive = 0.5 * losses.mean()

    l_nonneg = nonneg_sum / (B * B * NA * T * NV) / (Tf * Tf)
    log_t = np.log(Tf)
    temp_low = max(-log_t, 0.0) ** 4
    temp_high = max(log_t - np.log(3.0), 0.0) ** 4
    reg = l_nonneg + temp_low + temp_high
    total = contrastive + 0.3 * reg
    return (np.float32(total), np.float32(contrastive), np.float32(reg))


MM_DTYPE = "float8e4"


def _sim_scale(mm_dtype_name=None):
    name = MM_DTYPE if mm_dtype_name is None else mm_dtype_name
    return FP8_SCALE * FP8_SCALE if name.startswith("float8") else 1.0


def kernel(audio_feats, visual_feats, temperature):
    from concourse.bass_utils import run_bass_kernel_spmd

    nc = _get_program(MM_DTYPE)
    in_maps = _prep_inputs(audio_feats, visual_feats, MM_DTYPE)
    res = run_bass_kernel_spmd(nc, in_maps, list(range(N_CORES)))
    core_outs = [res.results[m] for m in range(N_CORES)]
    return _finalize(core_outs, temperature, _sim_scale())
